# revision 14
# baseline (speedup 1.0000x reference)
"""Trainium2 Bass kernel for ArccosHessianCalculator.

Math: for each batch element b (z1, z2 are [B, D] with D = 128):
  a = 1/|z1|, bb = 1/|z2|, c = cos = <z1u, z2u>
  Each Hessian block H_k is a rank-2 outer product plus a diagonal term:
      H_k(b) = z1 * r0_k(b)^T + z2 * r1_k(b)^T + diag-part
  where r0/r1 are per-element linear combinations of z1, z2 (all the
  normalization / cosine scale factors folded into the coefficients):
      k=0 (H11): r0 = -3c*a^4*z1 + a^3 b*z2          r1 = a^3 b*z1
      k=1 (H12): r0 = a^3 b*z1                        r1 = -c*a^2 b^2*z1 + a b^3*z2
      k=2 (H22): r0 = a b^3*z2                        r1 = a b^3*z1 - 3c*b^4*z2
  The (full, final) diagonals are computed separately in closed form and
  spliced in with a predicated copy against an identity mask.

Mapping to the chip (per core, batch shard of 512):
  - TensorE: one K=2 matmul per element, lhsT = [z1(b); z2(b)] ([2,128]),
    rhs = [r0 | r1] blocks ([2, 384]), streamed as float32r (1 cyc/row).
    Operands live at partition offsets {0,32,64,96} (tile_position rule).
  - ScalarE: PSUM -> SBUF staging copy.
  - VectorE: stats + rhs coefficient builds + diagonal splice
    (copy_predicated with an eye mask and a broadcast diagonal column).
  - DMA: batched 2MB output writes, one per (staging group, k).
Output per core: [3, 512, 128, 128] f32 (~100MB) -> DMA-bound overall.
"""

import numpy as np
from contextlib import ExitStack

import concourse.bass as bass
import concourse.tile as tile
from concourse import bacc, mybir
from concourse.bass_utils import run_bass_kernel_spmd

N_CORES = 8
B_FULL = 4096
D = 128
B_SH = B_FULL // N_CORES  # 512 batch elements per core
P = 128                   # SBUF partitions
KD = 3 * D                # 384: three H blocks side by side
F = 16                    # elements per partition-group in ZI/RI tiles (4*F per tile)
G = 32                    # elements per staging/out-DMA group
GROUPS = B_SH // P        # 4 stats groups of 128 elements

f32 = mybir.dt.float32
f32r = mybir.dt.float32r
i32 = mybir.dt.int32


def _build_body(ctx, tc, z1, z2, out, use_f32r=True):
    nc = tc.nc
    A = mybir.AluOpType

    const = ctx.enter_context(tc.tile_pool(name="const", bufs=1))
    zg_pool = ctx.enter_context(tc.tile_pool(name="zg", bufs=2))
    work = ctx.enter_context(tc.tile_pool(name="work", bufs=2))
    stat = ctx.enter_context(tc.tile_pool(name="stat", bufs=2))
    rpool = ctx.enter_context(tc.tile_pool(name="rpool", bufs=2))
    dpool = ctx.enter_context(tc.tile_pool(name="dpool", bufs=2))
    zi_pool = ctx.enter_context(tc.tile_pool(name="zi", bufs=2))
    ri_pool = ctx.enter_context(tc.tile_pool(name="ri", bufs=2))
    stage = ctx.enter_context(tc.tile_pool(name="stage", bufs=2))
    mmp = ctx.enter_context(tc.tile_pool(name="mmp", bufs=6, space="PSUM"))
    tpp = ctx.enter_context(tc.tile_pool(name="tpp", bufs=2, space="PSUM"))

    # --- constants: eye [128,128] and eye3 = [eye|eye|eye] ---
    colidx_i = const.tile([P, D], i32)
    nc.gpsimd.iota(colidx_i[:], [[1, D]], base=0, channel_multiplier=0)
    rowidx_i = const.tile([P, 1], i32)
    nc.gpsimd.iota(rowidx_i[:], [[0, 1]], base=0, channel_multiplier=1)
    colidx = const.tile([P, D], f32)
    nc.vector.tensor_copy(colidx[:], colidx_i[:])
    rowidx = const.tile([P, 1], f32)
    nc.vector.tensor_copy(rowidx[:], rowidx_i[:])
    eye = const.tile([P, D], f32)
    nc.vector.tensor_scalar(eye[:], colidx[:], rowidx[:], None, A.is_equal)
    # integer mask for copy_predicated (hw requires an int mask dtype)
    eyem = const.tile([P, D], mybir.dt.uint8)
    nc.vector.tensor_scalar(eyem[:], colidx[:], rowidx[:], None, A.is_equal)
    eye3 = const.tile([P, KD], mybir.dt.uint8)
    for k in range(3):
        nc.vector.tensor_copy(eye3[:, k * D:(k + 1) * D], eyem[:])

    for grp in range(GROUPS):
        b0 = grp * P

        z1g = zg_pool.tile([P, D], f32, tag="z1g")
        nc.sync.dma_start(z1g[:], z1[b0:b0 + P, :])
        z2g = zg_pool.tile([P, D], f32, tag="z2g")
        nc.sync.dma_start(z2g[:], z2[b0:b0 + P, :])

        v1z = work.tile([P, D], f32, tag="v1z")
        nc.vector.tensor_mul(v1z[:], z1g[:], z1g[:])
        v2z = work.tile([P, D], f32, tag="v2z")
        nc.vector.tensor_mul(v2z[:], z2g[:], z2g[:])
        wz = work.tile([P, D], f32, tag="wz")
        nc.vector.tensor_mul(wz[:], z1g[:], z2g[:])

        def sv(tag):
            return stat.tile([P, 1], f32, tag=tag, name=f"sv_{tag}_{grp}")

        s1 = sv("s1")
        nc.vector.reduce_sum(s1[:], v1z[:], axis=mybir.AxisListType.X)
        s2 = sv("s2")
        nc.vector.reduce_sum(s2[:], v2z[:], axis=mybir.AxisListType.X)
        dot = sv("dot")
        nc.vector.reduce_sum(dot[:], wz[:], axis=mybir.AxisListType.X)
        n1 = sv("n1")
        nc.scalar.sqrt(n1[:], s1[:])
        n2 = sv("n2")
        nc.scalar.sqrt(n2[:], s2[:])
        a = sv("a")
        nc.vector.reciprocal(a[:], n1[:])
        bb = sv("bb")
        nc.vector.reciprocal(bb[:], n2[:])
        a2 = sv("a2")
        nc.vector.tensor_mul(a2[:], a[:], a[:])
        b2 = sv("b2")
        nc.vector.tensor_mul(b2[:], bb[:], bb[:])
        ab = sv("ab")
        nc.vector.tensor_mul(ab[:], a[:], bb[:])
        c = sv("c")
        nc.vector.tensor_mul(c[:], dot[:], ab[:])
        m3c = sv("m3c")
        nc.vector.tensor_scalar(m3c[:], c[:], -3.0, None, A.mult)
        mc = sv("mc")
        nc.vector.tensor_scalar(mc[:], c[:], -1.0, None, A.mult)
        A3B = sv("A3B")
        nc.vector.tensor_mul(A3B[:], a2[:], ab[:])
        AB3 = sv("AB3")
        nc.vector.tensor_mul(AB3[:], b2[:], ab[:])
        A4 = sv("A4")
        nc.vector.tensor_mul(A4[:], a2[:], a2[:])
        B4 = sv("B4")
        nc.vector.tensor_mul(B4[:], b2[:], b2[:])
        A2B2 = sv("A2B2")
        nc.vector.tensor_mul(A2B2[:], ab[:], ab[:])
        m3cA4 = sv("m3cA4")
        nc.vector.tensor_mul(m3cA4[:], A4[:], m3c[:])
        m3cB4 = sv("m3cB4")
        nc.vector.tensor_mul(m3cB4[:], B4[:], m3c[:])
        mcA2B2 = sv("mcA2B2")
        nc.vector.tensor_mul(mcA2B2[:], A2B2[:], mc[:])
        mcab = sv("mcab")
        nc.vector.tensor_mul(mcab[:], ab[:], mc[:])

        # --- rhs rows R0, R1 [128b, 384] ---
        # (float32r: the DVE writes round-to-fp32r so the PE can stream them
        # single-pass; walrus rejects fp32r matmul inputs produced un-rounded)
        mmdt = f32r if use_f32r else f32
        R0 = rpool.tile([P, KD], mmdt, tag="R0")
        R1 = rpool.tile([P, KD], mmdt, tag="R1")
        t0 = work.tile([P, D], f32, tag="t0")
        # k=0 (H11): r0 = m3cA4*z1 + A3B*z2 ; r1 = A3B*z1
        nc.vector.tensor_scalar(t0[:], z2g[:], A3B[:], None, A.mult)
        nc.vector.scalar_tensor_tensor(
            R0[:, 0:D], z1g[:], m3cA4[:], t0[:], A.mult, A.add)
        nc.vector.tensor_scalar(R1[:, 0:D], z1g[:], A3B[:], None, A.mult)
        # k=1 (H12): r0 = A3B*z1 ; r1 = mcA2B2*z1 + AB3*z2
        nc.vector.tensor_scalar(R0[:, D:2 * D], z1g[:], A3B[:], None, A.mult)
        t1 = work.tile([P, D], f32, tag="t1")
        nc.vector.tensor_scalar(t1[:], z2g[:], AB3[:], None, A.mult)
        nc.vector.scalar_tensor_tensor(
            R1[:, D:2 * D], z1g[:], mcA2B2[:], t1[:], A.mult, A.add)
        # k=2 (H22): r0 = AB3*z2 ; r1 = AB3*z1 + m3cB4*z2
        nc.vector.tensor_scalar(R0[:, 2 * D:3 * D], z2g[:], AB3[:], None, A.mult)
        t2 = work.tile([P, D], f32, tag="t2")
        nc.vector.tensor_scalar(t2[:], z2g[:], m3cB4[:], None, A.mult)
        nc.vector.scalar_tensor_tensor(
            R1[:, 2 * D:3 * D], z1g[:], AB3[:], t2[:], A.mult, A.add)

        # --- final diagonal values, batch-major [128b, 128i] ---
        twoabw = work.tile([P, D], f32, tag="twoabw")
        nc.vector.tensor_scalar(twoabw[:], wz[:], ab[:], 2.0, A.mult, A.mult)
        # d11 = a2*(c + 2ab*wz + m3c*a2*v1z)
        u1 = work.tile([P, D], f32, tag="u1")
        nc.vector.tensor_scalar(u1[:], v1z[:], a2[:], m3c[:], A.mult, A.mult)
        u2 = work.tile([P, D], f32, tag="u2")
        nc.vector.tensor_add(u2[:], u1[:], twoabw[:])
        d11 = dpool.tile([P, D], f32, tag="d11")
        nc.vector.tensor_scalar(d11[:], u2[:], c[:], a2[:], A.add, A.mult)
        # d22 = b2*(c + 2ab*wz + m3c*b2*v2z)
        u3 = work.tile([P, D], f32, tag="u3")
        nc.vector.tensor_scalar(u3[:], v2z[:], b2[:], m3c[:], A.mult, A.mult)
        u4 = work.tile([P, D], f32, tag="u4")
        nc.vector.tensor_add(u4[:], u3[:], twoabw[:])
        d22 = dpool.tile([P, D], f32, tag="d22")
        nc.vector.tensor_scalar(d22[:], u4[:], c[:], b2[:], A.add, A.mult)
        # d12 = ab*(a2*v1z + b2*v2z + mcab*wz - 1)
        w1 = work.tile([P, D], f32, tag="w1")
        nc.vector.tensor_scalar(w1[:], v1z[:], a2[:], None, A.mult)
        w2 = work.tile([P, D], f32, tag="w2")
        nc.vector.scalar_tensor_tensor(w2[:], v2z[:], b2[:], w1[:], A.mult, A.add)
        w3 = work.tile([P, D], f32, tag="w3")
        nc.vector.scalar_tensor_tensor(w3[:], wz[:], mcab[:], w2[:], A.mult, A.add)
        d12 = dpool.tile([P, D], f32, tag="d12")
        nc.vector.tensor_scalar(d12[:], w3[:], -1.0, ab[:], A.add, A.mult)

        # --- rounded copies of z1/z2 for the matmul lhsT gathers ---
        z1r = zg_pool.tile([P, D], mmdt, tag="z1r")
        nc.vector.tensor_copy(z1r[:], z1g[:])
        z2r = zg_pool.tile([P, D], mmdt, tag="z2r")
        nc.vector.tensor_copy(z2r[:], z2g[:])

        # --- transpose diagonals into [128i, 3*128b] ---
        diagT = dpool.tile([P, KD], f32, tag="diagT")
        for k, dk in enumerate([d11, d12, d22]):
            pt = tpp.tile([P, D], f32, tag="tp", name=f"tp_{grp}_{k}")
            nc.tensor.transpose(pt[:], dk[:], eye[:])
            nc.scalar.copy(diagT[:, k * D:(k + 1) * D], pt[:])

        # --- chunks of 32 elements: 2 partition-groups (offsets 0/32) x F ---
        # (operand base partitions are limited to {0,32,64}; quadrant 96 is
        # unusable, so use 2 groups per tile)
        for ch in range(P // G):
            e0 = b0 + ch * G          # global element base for this chunk
            q0 = ch * G               # within-group base
            # Gather F batch rows into one partition row per (group, operand).
            # Out is a single-partition free-linear run; in is a plain slice —
            # stream orders match (b-major), dma_start only checks total size.
            ZI = zi_pool.tile([P, F * D], mmdt, tag="ZI", name=f"ZI_{grp}_{ch}")
            RI = ri_pool.tile([P, F * KD], mmdt, tag="RI", name=f"RI_{grp}_{ch}")
            for g in range(2):
                qs = q0 + g * F
                nc.sync.dma_start(ZI[32 * g:32 * g + 1, :], z1r[qs:qs + F, :])
                nc.sync.dma_start(ZI[32 * g + 1:32 * g + 2, :], z2r[qs:qs + F, :])
                nc.sync.dma_start(RI[32 * g:32 * g + 1, :], R0[qs:qs + F, :])
                nc.sync.dma_start(RI[32 * g + 1:32 * g + 2, :], R1[qs:qs + F, :])

            STG = stage.tile([P, G * KD], f32, tag="STG", name=f"STG_{grp}_{ch}")
            for s in range(G):
                q = q0 + s                # element idx within group (0..127)
                g4, ff = s // F, s % F
                pp = 32 * g4
                lhsT = ZI[pp:pp + 2, ff * D:(ff + 1) * D]
                rhs = RI[pp:pp + 2, ff * KD:(ff + 1) * KD]
                pt = mmp.tile([P, KD], f32, tag="pt", name=f"pt_{grp}_{ch}_{s}")
                nc.tensor.matmul(pt[:], lhsT, rhs, start=True, stop=True)
                dst = STG[:, s * KD:(s + 1) * KD]
                nc.scalar.copy(dst, pt[:])
                # (p, j, k) dim order keeps all three APs 3-D (unmergeable),
                # so their lowered shapes agree despite the broadcast dim.
                dstv = dst.rearrange("p (k j) -> p j k", k=3)
                maskv = eye3[:].rearrange("p (k j) -> p j k", k=3)
                datav = diagT[:].rearrange("p (k b) -> p b k", k=3)[
                    :, q:q + 1, :].broadcast_to([P, D, 3])
                nc.vector.copy_predicated(dstv, maskv, datav)
            stgv = STG[:].rearrange("p (e n) -> p e n", n=KD)
            for k in range(3):
                dram = out[k, e0:e0 + G, :, :].transpose([1, 0, 2])
                nc.sync.dma_start(dram, stgv[:, :, k * D:(k + 1) * D])


def build_kernel(use_f32r=True):
    nc = bacc.Bacc("TRN2", target_bir_lowering=False, debug=False)
    z1 = nc.dram_tensor("z1", [B_SH, D], f32, kind="ExternalInput").ap()
    z2 = nc.dram_tensor("z2", [B_SH, D], f32, kind="ExternalInput").ap()
    out = nc.dram_tensor("out", [3, B_SH, D, D], f32, kind="ExternalOutput").ap()
    with tile.TileContext(nc) as tc:
        with ExitStack() as ctx:
            _build_body(ctx, tc, z1, z2, out, use_f32r=use_f32r)
    nc.compile()
    return nc


_NC_CACHE = None


def _get_nc():
    global _NC_CACHE
    if _NC_CACHE is None:
        _NC_CACHE = build_kernel()
    return _NC_CACHE


def kernel(z1, z2):
    nc = _get_nc()
    z1 = np.ascontiguousarray(np.asarray(z1, dtype=np.float32))
    z2 = np.ascontiguousarray(np.asarray(z2, dtype=np.float32))
    in_maps = [
        {"z1": z1[c * B_SH:(c + 1) * B_SH], "z2": z2[c * B_SH:(c + 1) * B_SH]}
        for c in range(N_CORES)
    ]
    res = run_bass_kernel_spmd(nc, in_maps, core_ids=list(range(N_CORES)))
    return np.concatenate([res.results[c]["out"] for c in range(N_CORES)], axis=1)


# revision 15
# speedup vs baseline: 1.9214x; 1.9214x over previous
"""Trainium2 Bass kernel for ArccosHessianCalculator.

Math: for each batch element b (z1, z2 are [B, D] with D = 128):
  a = 1/|z1|, bb = 1/|z2|, c = cos = <z1u, z2u>
  Each Hessian block H_k is a rank-2 outer product plus a diagonal term:
      H_k(b) = z1 * r0_k(b)^T + z2 * r1_k(b)^T + diag-part
  where r0/r1 are per-element linear combinations of z1, z2 (all the
  normalization / cosine scale factors folded into the coefficients):
      k=0 (H11): r0 = -3c*a^4*z1 + a^3 b*z2          r1 = a^3 b*z1
      k=1 (H12): r0 = a^3 b*z1                        r1 = -c*a^2 b^2*z1 + a b^3*z2
      k=2 (H22): r0 = a b^3*z2                        r1 = a b^3*z1 - 3c*b^4*z2
  The (full, final) diagonals are computed separately in closed form and
  spliced in with a predicated copy against an identity mask.

Mapping to the chip (per core, batch shard of 512):
  - TensorE: one K=2 matmul per element, lhsT = [z1(b); z2(b)] ([2,128]),
    rhs = [r0 | r1] blocks ([2, 384]), streamed as float32r (1 cyc/row).
    Operands live at partition offsets {0,32,64,96} (tile_position rule).
  - ScalarE: PSUM -> SBUF staging copy.
  - VectorE: stats + rhs coefficient builds + diagonal splice
    (copy_predicated with an eye mask and a broadcast diagonal column).
  - DMA: batched 2MB output writes, one per (staging group, k).
Output per core: [3, 512, 128, 128] f32 (~100MB) -> DMA-bound overall.
"""

import numpy as np
from contextlib import ExitStack

import concourse.bass as bass
import concourse.tile as tile
from concourse import bacc, mybir
from concourse.bass_utils import run_bass_kernel_spmd

N_CORES = 8
B_FULL = 4096
D = 128
B_SH = B_FULL // N_CORES  # 512 batch elements per core
P = 128                   # SBUF partitions
KD = 3 * D                # 384: three H blocks side by side
F = 16                    # elements per partition-group in ZI/RI tiles (4*F per tile)
G = 32                    # elements per staging/out-DMA group
GROUPS = B_SH // P        # 4 stats groups of 128 elements

f32 = mybir.dt.float32
f32r = mybir.dt.float32r
i32 = mybir.dt.int32


def _build_body(ctx, tc, z1, z2, out, use_f32r=True):
    nc = tc.nc
    A = mybir.AluOpType

    const = ctx.enter_context(tc.tile_pool(name="const", bufs=1))
    zg_pool = ctx.enter_context(tc.tile_pool(name="zg", bufs=2))
    work = ctx.enter_context(tc.tile_pool(name="work", bufs=2))
    stat = ctx.enter_context(tc.tile_pool(name="stat", bufs=2))
    rpool = ctx.enter_context(tc.tile_pool(name="rpool", bufs=2))
    dpool = ctx.enter_context(tc.tile_pool(name="dpool", bufs=2))
    zi_pool = ctx.enter_context(tc.tile_pool(name="zi", bufs=2))
    ri_pool = ctx.enter_context(tc.tile_pool(name="ri", bufs=2))
    stage = ctx.enter_context(tc.tile_pool(name="stage", bufs=2))
    mmp = ctx.enter_context(tc.tile_pool(name="mmp", bufs=6, space="PSUM"))
    tpp = ctx.enter_context(tc.tile_pool(name="tpp", bufs=2, space="PSUM"))

    # --- constants: eye [128,128] and eye3 = [eye|eye|eye] ---
    colidx_i = const.tile([P, D], i32)
    nc.gpsimd.iota(colidx_i[:], [[1, D]], base=0, channel_multiplier=0)
    rowidx_i = const.tile([P, 1], i32)
    nc.gpsimd.iota(rowidx_i[:], [[0, 1]], base=0, channel_multiplier=1)
    colidx = const.tile([P, D], f32)
    nc.vector.tensor_copy(colidx[:], colidx_i[:])
    rowidx = const.tile([P, 1], f32)
    nc.vector.tensor_copy(rowidx[:], rowidx_i[:])
    eye = const.tile([P, D], f32)
    nc.vector.tensor_scalar(eye[:], colidx[:], rowidx[:], None, A.is_equal)
    # integer mask for copy_predicated (hw requires an int mask dtype)
    eyem = const.tile([P, D], mybir.dt.uint8)
    nc.vector.tensor_scalar(eyem[:], colidx[:], rowidx[:], None, A.is_equal)
    eye3 = const.tile([P, KD], mybir.dt.uint8)
    for k in range(3):
        nc.vector.tensor_copy(eye3[:, k * D:(k + 1) * D], eyem[:])

    for grp in range(GROUPS):
        b0 = grp * P

        z1g = zg_pool.tile([P, D], f32, tag="z1g")
        nc.sync.dma_start(z1g[:], z1[b0:b0 + P, :])
        z2g = zg_pool.tile([P, D], f32, tag="z2g")
        nc.sync.dma_start(z2g[:], z2[b0:b0 + P, :])

        v1z = work.tile([P, D], f32, tag="v1z")
        nc.vector.tensor_mul(v1z[:], z1g[:], z1g[:])
        v2z = work.tile([P, D], f32, tag="v2z")
        nc.vector.tensor_mul(v2z[:], z2g[:], z2g[:])
        wz = work.tile([P, D], f32, tag="wz")
        nc.vector.tensor_mul(wz[:], z1g[:], z2g[:])

        def sv(tag):
            return stat.tile([P, 1], f32, tag=tag, name=f"sv_{tag}_{grp}")

        s1 = sv("s1")
        nc.vector.reduce_sum(s1[:], v1z[:], axis=mybir.AxisListType.X)
        s2 = sv("s2")
        nc.vector.reduce_sum(s2[:], v2z[:], axis=mybir.AxisListType.X)
        dot = sv("dot")
        nc.vector.reduce_sum(dot[:], wz[:], axis=mybir.AxisListType.X)
        n1 = sv("n1")
        nc.scalar.sqrt(n1[:], s1[:])
        n2 = sv("n2")
        nc.scalar.sqrt(n2[:], s2[:])
        a = sv("a")
        nc.vector.reciprocal(a[:], n1[:])
        bb = sv("bb")
        nc.vector.reciprocal(bb[:], n2[:])
        a2 = sv("a2")
        nc.vector.tensor_mul(a2[:], a[:], a[:])
        b2 = sv("b2")
        nc.vector.tensor_mul(b2[:], bb[:], bb[:])
        ab = sv("ab")
        nc.vector.tensor_mul(ab[:], a[:], bb[:])
        c = sv("c")
        nc.vector.tensor_mul(c[:], dot[:], ab[:])
        m3c = sv("m3c")
        nc.vector.tensor_scalar(m3c[:], c[:], -3.0, None, A.mult)
        mc = sv("mc")
        nc.vector.tensor_scalar(mc[:], c[:], -1.0, None, A.mult)
        A3B = sv("A3B")
        nc.vector.tensor_mul(A3B[:], a2[:], ab[:])
        AB3 = sv("AB3")
        nc.vector.tensor_mul(AB3[:], b2[:], ab[:])
        A4 = sv("A4")
        nc.vector.tensor_mul(A4[:], a2[:], a2[:])
        B4 = sv("B4")
        nc.vector.tensor_mul(B4[:], b2[:], b2[:])
        A2B2 = sv("A2B2")
        nc.vector.tensor_mul(A2B2[:], ab[:], ab[:])
        m3cA4 = sv("m3cA4")
        nc.vector.tensor_mul(m3cA4[:], A4[:], m3c[:])
        m3cB4 = sv("m3cB4")
        nc.vector.tensor_mul(m3cB4[:], B4[:], m3c[:])
        mcA2B2 = sv("mcA2B2")
        nc.vector.tensor_mul(mcA2B2[:], A2B2[:], mc[:])
        mcab = sv("mcab")
        nc.vector.tensor_mul(mcab[:], ab[:], mc[:])

        # --- rhs rows R0, R1 [128b, 384] ---
        # (float32r: the DVE writes round-to-fp32r so the PE can stream them
        # single-pass; walrus rejects fp32r matmul inputs produced un-rounded)
        mmdt = f32r if use_f32r else f32
        R0 = rpool.tile([P, KD], mmdt, tag="R0")
        R1 = rpool.tile([P, KD], mmdt, tag="R1")
        t0 = work.tile([P, D], f32, tag="t0")
        # k=0 (H11): r0 = m3cA4*z1 + A3B*z2 ; r1 = A3B*z1
        nc.vector.tensor_scalar(t0[:], z2g[:], A3B[:], None, A.mult)
        nc.vector.scalar_tensor_tensor(
            R0[:, 0:D], z1g[:], m3cA4[:], t0[:], A.mult, A.add)
        nc.vector.tensor_scalar(R1[:, 0:D], z1g[:], A3B[:], None, A.mult)
        # k=1 (H12): r0 = A3B*z1 ; r1 = mcA2B2*z1 + AB3*z2
        nc.vector.tensor_scalar(R0[:, D:2 * D], z1g[:], A3B[:], None, A.mult)
        t1 = work.tile([P, D], f32, tag="t1")
        nc.vector.tensor_scalar(t1[:], z2g[:], AB3[:], None, A.mult)
        nc.vector.scalar_tensor_tensor(
            R1[:, D:2 * D], z1g[:], mcA2B2[:], t1[:], A.mult, A.add)
        # k=2 (H22): r0 = AB3*z2 ; r1 = AB3*z1 + m3cB4*z2
        nc.vector.tensor_scalar(R0[:, 2 * D:3 * D], z2g[:], AB3[:], None, A.mult)
        t2 = work.tile([P, D], f32, tag="t2")
        nc.vector.tensor_scalar(t2[:], z2g[:], m3cB4[:], None, A.mult)
        nc.vector.scalar_tensor_tensor(
            R1[:, 2 * D:3 * D], z1g[:], AB3[:], t2[:], A.mult, A.add)

        # --- final diagonal values, batch-major [128b, 128i] ---
        twoabw = work.tile([P, D], f32, tag="twoabw")
        nc.vector.tensor_scalar(twoabw[:], wz[:], ab[:], 2.0, A.mult, A.mult)
        # d11 = a2*(c + 2ab*wz + m3c*a2*v1z)
        u1 = work.tile([P, D], f32, tag="u1")
        nc.vector.tensor_scalar(u1[:], v1z[:], a2[:], m3c[:], A.mult, A.mult)
        u2 = work.tile([P, D], f32, tag="u2")
        nc.vector.tensor_add(u2[:], u1[:], twoabw[:])
        d11 = dpool.tile([P, D], f32, tag="d11")
        nc.vector.tensor_scalar(d11[:], u2[:], c[:], a2[:], A.add, A.mult)
        # d22 = b2*(c + 2ab*wz + m3c*b2*v2z)
        u3 = work.tile([P, D], f32, tag="u3")
        nc.vector.tensor_scalar(u3[:], v2z[:], b2[:], m3c[:], A.mult, A.mult)
        u4 = work.tile([P, D], f32, tag="u4")
        nc.vector.tensor_add(u4[:], u3[:], twoabw[:])
        d22 = dpool.tile([P, D], f32, tag="d22")
        nc.vector.tensor_scalar(d22[:], u4[:], c[:], b2[:], A.add, A.mult)
        # d12 = ab*(a2*v1z + b2*v2z + mcab*wz - 1)
        w1 = work.tile([P, D], f32, tag="w1")
        nc.vector.tensor_scalar(w1[:], v1z[:], a2[:], None, A.mult)
        w2 = work.tile([P, D], f32, tag="w2")
        nc.vector.scalar_tensor_tensor(w2[:], v2z[:], b2[:], w1[:], A.mult, A.add)
        w3 = work.tile([P, D], f32, tag="w3")
        nc.vector.scalar_tensor_tensor(w3[:], wz[:], mcab[:], w2[:], A.mult, A.add)
        d12 = dpool.tile([P, D], f32, tag="d12")
        nc.vector.tensor_scalar(d12[:], w3[:], -1.0, ab[:], A.add, A.mult)

        # --- rounded copies of z1/z2 for the matmul lhsT gathers ---
        z1r = zg_pool.tile([P, D], mmdt, tag="z1r")
        nc.vector.tensor_copy(z1r[:], z1g[:])
        z2r = zg_pool.tile([P, D], mmdt, tag="z2r")
        nc.vector.tensor_copy(z2r[:], z2g[:])

        # --- transpose diagonals into [128i, 3*128b] ---
        diagT = dpool.tile([P, KD], f32, tag="diagT")
        for k, dk in enumerate([d11, d12, d22]):
            pt = tpp.tile([P, D], f32, tag="tp", name=f"tp_{grp}_{k}")
            nc.tensor.transpose(pt[:], dk[:], eye[:])
            nc.scalar.copy(diagT[:, k * D:(k + 1) * D], pt[:])

        # --- chunks of 32 elements: 2 partition-groups (offsets 0/32) x F ---
        # (operand base partitions are limited to {0,32,64}; quadrant 96 is
        # unusable, so use 2 groups per tile)
        for ch in range(P // G):
            e0 = b0 + ch * G          # global element base for this chunk
            q0 = ch * G               # within-group base
            # Gather F batch rows into one partition row per (group, operand).
            # Out is a single-partition free-linear run; in is a plain slice —
            # stream orders match (b-major), dma_start only checks total size.
            ZI = zi_pool.tile([P, F * D], mmdt, tag="ZI", name=f"ZI_{grp}_{ch}")
            RI = ri_pool.tile([P, F * KD], mmdt, tag="RI", name=f"RI_{grp}_{ch}")
            for g in range(2):
                qs = q0 + g * F
                nc.sync.dma_start(ZI[32 * g:32 * g + 1, :], z1r[qs:qs + F, :])
                nc.sync.dma_start(ZI[32 * g + 1:32 * g + 2, :], z2r[qs:qs + F, :])
                nc.sync.dma_start(RI[32 * g:32 * g + 1, :], R0[qs:qs + F, :])
                nc.sync.dma_start(RI[32 * g + 1:32 * g + 2, :], R1[qs:qs + F, :])

            STG = stage.tile([P, G * KD], f32, tag="STG", name=f"STG_{grp}_{ch}")
            for s in range(G):
                q = q0 + s                # element idx within group (0..127)
                g4, ff = s // F, s % F
                pp = 32 * g4
                lhsT = ZI[pp:pp + 2, ff * D:(ff + 1) * D]
                rhs = RI[pp:pp + 2, ff * KD:(ff + 1) * KD]
                pt = mmp.tile([P, KD], f32, tag="pt", name=f"pt_{grp}_{ch}_{s}")
                nc.tensor.matmul(pt[:], lhsT, rhs, start=True, stop=True)
                dst = STG[:, s * KD:(s + 1) * KD]
                nc.scalar.copy(dst, pt[:])
                # contiguous innermost j on dst/mask; data is a per-k column
                # of diagT broadcast along j (step-0 inner dim)
                datav = diagT[:].rearrange("p (k b) -> p k b", k=3)[
                    :, :, q:q + 1].broadcast_to([P, 3, D])
                nc.vector.copy_predicated(dst, eye3[:], datav)
            stgv = STG[:].rearrange("p (e n) -> p e n", n=KD)
            for k in range(3):
                dram = out[k, e0:e0 + G, :, :].transpose([1, 0, 2])
                nc.sync.dma_start(dram, stgv[:, :, k * D:(k + 1) * D])


def build_kernel(use_f32r=True):
    nc = bacc.Bacc("TRN2", target_bir_lowering=False, debug=False)
    z1 = nc.dram_tensor("z1", [B_SH, D], f32, kind="ExternalInput").ap()
    z2 = nc.dram_tensor("z2", [B_SH, D], f32, kind="ExternalInput").ap()
    out = nc.dram_tensor("out", [3, B_SH, D, D], f32, kind="ExternalOutput").ap()
    with tile.TileContext(nc) as tc:
        with ExitStack() as ctx:
            _build_body(ctx, tc, z1, z2, out, use_f32r=use_f32r)
    nc.compile()
    return nc


_NC_CACHE = None


def _get_nc():
    global _NC_CACHE
    if _NC_CACHE is None:
        _NC_CACHE = build_kernel()
    return _NC_CACHE


def kernel(z1, z2):
    nc = _get_nc()
    z1 = np.ascontiguousarray(np.asarray(z1, dtype=np.float32))
    z2 = np.ascontiguousarray(np.asarray(z2, dtype=np.float32))
    in_maps = [
        {"z1": z1[c * B_SH:(c + 1) * B_SH], "z2": z2[c * B_SH:(c + 1) * B_SH]}
        for c in range(N_CORES)
    ]
    res = run_bass_kernel_spmd(nc, in_maps, core_ids=list(range(N_CORES)))
    return np.concatenate([res.results[c]["out"] for c in range(N_CORES)], axis=1)


# revision 18
# speedup vs baseline: 3.2022x; 1.6665x over previous
"""Trainium2 Bass kernel for ArccosHessianCalculator.

Math: for each batch element b (z1, z2 are [B, D] with D = 128):
  a = 1/|z1|, bb = 1/|z2|, c = cos = <z1u, z2u>
  Each Hessian block H_k is a rank-2 outer product plus a diagonal term:
      H_k(b) = z1 * r0_k(b)^T + z2 * r1_k(b)^T + diag-part
  where r0/r1 are per-element linear combinations of z1, z2 (all the
  normalization / cosine scale factors folded into the coefficients):
      k=0 (H11): r0 = -3c*a^4*z1 + a^3 b*z2          r1 = a^3 b*z1
      k=1 (H12): r0 = a^3 b*z1                        r1 = -c*a^2 b^2*z1 + a b^3*z2
      k=2 (H22): r0 = a b^3*z2                        r1 = a b^3*z1 - 3c*b^4*z2
  The (full, final) diagonals are computed separately in closed form and
  spliced in with a predicated copy against an identity mask.

Mapping to the chip (per core, batch shard of 512):
  - TensorE: one K=2 matmul per element, lhsT = [z1(b); z2(b)] ([2,128]),
    rhs = [r0 | r1] blocks ([2, 384]), streamed as float32r (1 cyc/row).
    Operands live at partition offsets {0,32,64,96} (tile_position rule).
  - ScalarE: PSUM -> SBUF staging copy.
  - VectorE: stats + rhs coefficient builds + diagonal splice
    (copy_predicated with an eye mask and a broadcast diagonal column).
  - DMA: batched 2MB output writes, one per (staging group, k).
Output per core: [3, 512, 128, 128] f32 (~100MB) -> DMA-bound overall.
"""

import numpy as np
from contextlib import ExitStack

import concourse.bass as bass
import concourse.tile as tile
from concourse import bacc, mybir
from concourse.bass_utils import run_bass_kernel_spmd

N_CORES = 8
B_FULL = 4096
D = 128
B_SH = B_FULL // N_CORES  # 512 batch elements per core
P = 128                   # SBUF partitions
KD = 3 * D                # 384: three H blocks side by side
F = 16                    # elements per partition-group in ZI/RI tiles (4*F per tile)
G = 32                    # elements per staging/out-DMA group
GROUPS = B_SH // P        # 4 stats groups of 128 elements

f32 = mybir.dt.float32
f32r = mybir.dt.float32r
i32 = mybir.dt.int32


def _build_body(ctx, tc, z1, z2, out, use_f32r=True):
    nc = tc.nc
    A = mybir.AluOpType

    const = ctx.enter_context(tc.tile_pool(name="const", bufs=1))
    zg_pool = ctx.enter_context(tc.tile_pool(name="zg", bufs=2))
    work = ctx.enter_context(tc.tile_pool(name="work", bufs=2))
    stat = ctx.enter_context(tc.tile_pool(name="stat", bufs=2))
    rpool = ctx.enter_context(tc.tile_pool(name="rpool", bufs=2))
    dpool = ctx.enter_context(tc.tile_pool(name="dpool", bufs=2))
    zi_pool = ctx.enter_context(tc.tile_pool(name="zi", bufs=2))
    ri_pool = ctx.enter_context(tc.tile_pool(name="ri", bufs=2))
    stage = ctx.enter_context(tc.tile_pool(name="stage", bufs=2))
    mmp = ctx.enter_context(tc.tile_pool(name="mmp", bufs=6, space="PSUM"))
    tpp = ctx.enter_context(tc.tile_pool(name="tpp", bufs=2, space="PSUM"))

    # --- constants: eye [128,128] and eye3 = [eye|eye|eye] ---
    colidx_i = const.tile([P, D], i32)
    nc.gpsimd.iota(colidx_i[:], [[1, D]], base=0, channel_multiplier=0)
    rowidx_i = const.tile([P, 1], i32)
    nc.gpsimd.iota(rowidx_i[:], [[0, 1]], base=0, channel_multiplier=1)
    colidx = const.tile([P, D], f32)
    nc.vector.tensor_copy(colidx[:], colidx_i[:])
    rowidx = const.tile([P, 1], f32)
    nc.vector.tensor_copy(rowidx[:], rowidx_i[:])
    eye = const.tile([P, D], f32)
    nc.vector.tensor_scalar(eye[:], colidx[:], rowidx[:], None, A.is_equal)
    # integer mask for copy_predicated (hw requires an int mask dtype)
    eyem = const.tile([P, D], mybir.dt.uint8)
    nc.vector.tensor_scalar(eyem[:], colidx[:], rowidx[:], None, A.is_equal)
    eye3 = const.tile([P, KD], mybir.dt.uint8)
    for k in range(3):
        nc.vector.tensor_copy(eye3[:, k * D:(k + 1) * D], eyem[:])

    for grp in range(GROUPS):
        b0 = grp * P

        z1g = zg_pool.tile([P, D], f32, tag="z1g")
        nc.sync.dma_start(z1g[:], z1[b0:b0 + P, :])
        z2g = zg_pool.tile([P, D], f32, tag="z2g")
        nc.sync.dma_start(z2g[:], z2[b0:b0 + P, :])

        v1z = work.tile([P, D], f32, tag="v1z")
        nc.vector.tensor_mul(v1z[:], z1g[:], z1g[:])
        v2z = work.tile([P, D], f32, tag="v2z")
        nc.vector.tensor_mul(v2z[:], z2g[:], z2g[:])
        wz = work.tile([P, D], f32, tag="wz")
        nc.vector.tensor_mul(wz[:], z1g[:], z2g[:])

        def sv(tag):
            return stat.tile([P, 1], f32, tag=tag, name=f"sv_{tag}_{grp}")

        s1 = sv("s1")
        nc.vector.reduce_sum(s1[:], v1z[:], axis=mybir.AxisListType.X)
        s2 = sv("s2")
        nc.vector.reduce_sum(s2[:], v2z[:], axis=mybir.AxisListType.X)
        dot = sv("dot")
        nc.vector.reduce_sum(dot[:], wz[:], axis=mybir.AxisListType.X)
        n1 = sv("n1")
        nc.scalar.sqrt(n1[:], s1[:])
        n2 = sv("n2")
        nc.scalar.sqrt(n2[:], s2[:])
        a = sv("a")
        nc.vector.reciprocal(a[:], n1[:])
        bb = sv("bb")
        nc.vector.reciprocal(bb[:], n2[:])
        a2 = sv("a2")
        nc.vector.tensor_mul(a2[:], a[:], a[:])
        b2 = sv("b2")
        nc.vector.tensor_mul(b2[:], bb[:], bb[:])
        ab = sv("ab")
        nc.vector.tensor_mul(ab[:], a[:], bb[:])
        c = sv("c")
        nc.vector.tensor_mul(c[:], dot[:], ab[:])
        m3c = sv("m3c")
        nc.vector.tensor_scalar(m3c[:], c[:], -3.0, None, A.mult)
        mc = sv("mc")
        nc.vector.tensor_scalar(mc[:], c[:], -1.0, None, A.mult)
        A3B = sv("A3B")
        nc.vector.tensor_mul(A3B[:], a2[:], ab[:])
        AB3 = sv("AB3")
        nc.vector.tensor_mul(AB3[:], b2[:], ab[:])
        A4 = sv("A4")
        nc.vector.tensor_mul(A4[:], a2[:], a2[:])
        B4 = sv("B4")
        nc.vector.tensor_mul(B4[:], b2[:], b2[:])
        A2B2 = sv("A2B2")
        nc.vector.tensor_mul(A2B2[:], ab[:], ab[:])
        m3cA4 = sv("m3cA4")
        nc.vector.tensor_mul(m3cA4[:], A4[:], m3c[:])
        m3cB4 = sv("m3cB4")
        nc.vector.tensor_mul(m3cB4[:], B4[:], m3c[:])
        mcA2B2 = sv("mcA2B2")
        nc.vector.tensor_mul(mcA2B2[:], A2B2[:], mc[:])
        mcab = sv("mcab")
        nc.vector.tensor_mul(mcab[:], ab[:], mc[:])

        # --- rhs rows R0, R1 [128b, 384] ---
        # (float32r: the DVE writes round-to-fp32r so the PE can stream them
        # single-pass; walrus rejects fp32r matmul inputs produced un-rounded)
        mmdt = f32r if use_f32r else f32
        R0 = rpool.tile([P, KD], mmdt, tag="R0")
        R1 = rpool.tile([P, KD], mmdt, tag="R1")
        t0 = work.tile([P, D], f32, tag="t0")
        # k=0 (H11): r0 = m3cA4*z1 + A3B*z2 ; r1 = A3B*z1
        nc.vector.tensor_scalar(t0[:], z2g[:], A3B[:], None, A.mult)
        nc.vector.scalar_tensor_tensor(
            R0[:, 0:D], z1g[:], m3cA4[:], t0[:], A.mult, A.add)
        nc.vector.tensor_scalar(R1[:, 0:D], z1g[:], A3B[:], None, A.mult)
        # k=1 (H12): r0 = A3B*z1 ; r1 = mcA2B2*z1 + AB3*z2
        nc.vector.tensor_scalar(R0[:, D:2 * D], z1g[:], A3B[:], None, A.mult)
        t1 = work.tile([P, D], f32, tag="t1")
        nc.vector.tensor_scalar(t1[:], z2g[:], AB3[:], None, A.mult)
        nc.vector.scalar_tensor_tensor(
            R1[:, D:2 * D], z1g[:], mcA2B2[:], t1[:], A.mult, A.add)
        # k=2 (H22): r0 = AB3*z2 ; r1 = AB3*z1 + m3cB4*z2
        nc.vector.tensor_scalar(R0[:, 2 * D:3 * D], z2g[:], AB3[:], None, A.mult)
        t2 = work.tile([P, D], f32, tag="t2")
        nc.vector.tensor_scalar(t2[:], z2g[:], m3cB4[:], None, A.mult)
        nc.vector.scalar_tensor_tensor(
            R1[:, 2 * D:3 * D], z1g[:], AB3[:], t2[:], A.mult, A.add)

        # --- final diagonal values, batch-major [128b, 128i] ---
        twoabw = work.tile([P, D], f32, tag="twoabw")
        nc.vector.tensor_scalar(twoabw[:], wz[:], ab[:], 2.0, A.mult, A.mult)
        # d11 = a2*(c + 2ab*wz + m3c*a2*v1z)
        u1 = work.tile([P, D], f32, tag="u1")
        nc.vector.tensor_scalar(u1[:], v1z[:], a2[:], m3c[:], A.mult, A.mult)
        u2 = work.tile([P, D], f32, tag="u2")
        nc.vector.tensor_add(u2[:], u1[:], twoabw[:])
        d11 = dpool.tile([P, D], f32, tag="d11")
        nc.vector.tensor_scalar(d11[:], u2[:], c[:], a2[:], A.add, A.mult)
        # d22 = b2*(c + 2ab*wz + m3c*b2*v2z)
        u3 = work.tile([P, D], f32, tag="u3")
        nc.vector.tensor_scalar(u3[:], v2z[:], b2[:], m3c[:], A.mult, A.mult)
        u4 = work.tile([P, D], f32, tag="u4")
        nc.vector.tensor_add(u4[:], u3[:], twoabw[:])
        d22 = dpool.tile([P, D], f32, tag="d22")
        nc.vector.tensor_scalar(d22[:], u4[:], c[:], b2[:], A.add, A.mult)
        # d12 = ab*(a2*v1z + b2*v2z + mcab*wz - 1)
        w1 = work.tile([P, D], f32, tag="w1")
        nc.vector.tensor_scalar(w1[:], v1z[:], a2[:], None, A.mult)
        w2 = work.tile([P, D], f32, tag="w2")
        nc.vector.scalar_tensor_tensor(w2[:], v2z[:], b2[:], w1[:], A.mult, A.add)
        w3 = work.tile([P, D], f32, tag="w3")
        nc.vector.scalar_tensor_tensor(w3[:], wz[:], mcab[:], w2[:], A.mult, A.add)
        d12 = dpool.tile([P, D], f32, tag="d12")
        nc.vector.tensor_scalar(d12[:], w3[:], -1.0, ab[:], A.add, A.mult)

        # --- rounded copies of z1/z2 for the matmul lhsT gathers ---
        z1r = zg_pool.tile([P, D], mmdt, tag="z1r")
        nc.vector.tensor_copy(z1r[:], z1g[:])
        z2r = zg_pool.tile([P, D], mmdt, tag="z2r")
        nc.vector.tensor_copy(z2r[:], z2g[:])

        # --- transpose diagonals into [128i, 3*128b] ---
        diagT = dpool.tile([P, KD], f32, tag="diagT")
        for k, dk in enumerate([d11, d12, d22]):
            pt = tpp.tile([P, D], f32, tag="tp", name=f"tp_{grp}_{k}")
            nc.tensor.transpose(pt[:], dk[:], eye[:])
            nc.scalar.copy(diagT[:, k * D:(k + 1) * D], pt[:])

        # --- chunks of 32 elements: 2 partition-groups (offsets 0/32) x F ---
        # (operand base partitions are limited to {0,32,64}; quadrant 96 is
        # unusable, so use 2 groups per tile)
        for ch in range(P // G):
            e0 = b0 + ch * G          # global element base for this chunk
            q0 = ch * G               # within-group base
            # Gather F batch rows into one partition row per (group, operand).
            # Out is a single-partition free-linear run; in is a plain slice —
            # stream orders match (b-major), dma_start only checks total size.
            ZI = zi_pool.tile([P, F * D], mmdt, tag="ZI", name=f"ZI_{grp}_{ch}")
            RI = ri_pool.tile([P, F * KD], mmdt, tag="RI", name=f"RI_{grp}_{ch}")
            # gathers ride the (otherwise idle) gpsimd SWDGE path so the big
            # output writes on the sync HWDGE ring can't head-of-line block
            # the next chunk's operands
            for g in range(2):
                qs = q0 + g * F
                nc.gpsimd.dma_start(ZI[32 * g:32 * g + 1, :], z1r[qs:qs + F, :])
                nc.gpsimd.dma_start(ZI[32 * g + 1:32 * g + 2, :], z2r[qs:qs + F, :])
                nc.gpsimd.dma_start(RI[32 * g:32 * g + 1, :], R0[qs:qs + F, :])
                nc.gpsimd.dma_start(RI[32 * g + 1:32 * g + 2, :], R1[qs:qs + F, :])

            STG = stage.tile([P, G * KD], f32, tag="STG", name=f"STG_{grp}_{ch}")
            for s in range(G):
                q = q0 + s                # element idx within group (0..127)
                g4, ff = s // F, s % F
                pp = 32 * g4
                lhsT = ZI[pp:pp + 2, ff * D:(ff + 1) * D]
                rhs = RI[pp:pp + 2, ff * KD:(ff + 1) * KD]
                pt = mmp.tile([P, KD], f32, tag="pt", name=f"pt_{grp}_{ch}_{s}")
                nc.tensor.matmul(pt[:], lhsT, rhs, start=True, stop=True)
                dst = STG[:, s * KD:(s + 1) * KD]
                nc.scalar.copy(dst, pt[:])
                # contiguous innermost j on dst/mask; data is a per-k column
                # of diagT broadcast along j (step-0 inner dim)
                datav = diagT[:].rearrange("p (k b) -> p k b", k=3)[
                    :, :, q:q + 1].broadcast_to([P, 3, D])
                nc.vector.copy_predicated(dst, eye3[:], datav)
            stgv = STG[:].rearrange("p (e n) -> p e n", n=KD)
            for k in range(3):
                dram = out[k, e0:e0 + G, :, :].transpose([1, 0, 2])
                nc.sync.dma_start(dram, stgv[:, :, k * D:(k + 1) * D])


def build_kernel(use_f32r=True):
    nc = bacc.Bacc("TRN2", target_bir_lowering=False, debug=False)
    z1 = nc.dram_tensor("z1", [B_SH, D], f32, kind="ExternalInput").ap()
    z2 = nc.dram_tensor("z2", [B_SH, D], f32, kind="ExternalInput").ap()
    out = nc.dram_tensor("out", [3, B_SH, D, D], f32, kind="ExternalOutput").ap()
    with tile.TileContext(nc) as tc:
        with ExitStack() as ctx:
            _build_body(ctx, tc, z1, z2, out, use_f32r=use_f32r)
    nc.compile()
    return nc


_NC_CACHE = None


def _get_nc():
    global _NC_CACHE
    if _NC_CACHE is None:
        _NC_CACHE = build_kernel()
    return _NC_CACHE


def kernel(z1, z2):
    nc = _get_nc()
    z1 = np.ascontiguousarray(np.asarray(z1, dtype=np.float32))
    z2 = np.ascontiguousarray(np.asarray(z2, dtype=np.float32))
    in_maps = [
        {"z1": z1[c * B_SH:(c + 1) * B_SH], "z2": z2[c * B_SH:(c + 1) * B_SH]}
        for c in range(N_CORES)
    ]
    res = run_bass_kernel_spmd(nc, in_maps, core_ids=list(range(N_CORES)))
    return np.concatenate([res.results[c]["out"] for c in range(N_CORES)], axis=1)
